# revision 5
# baseline (speedup 1.0000x reference)
"""Trainium2 kernel for nn_Dec_module_74285754351968 (FD_FEM: fft-split + MoE convs).

Mathematical structure exploited:
  - fd_split's fft/ifft roundtrip cancels exactly: high = |x|, low = 0.
  - FEM-b therefore sees an all-zero input -> its output is batch-independent
    and piecewise-constant over 9 border classes; computed on host in closed
    form (no convs needed).
  - FEM-a's gate (softmax over spatial means of conv1 output) is computed on
    host exactly from region sums of |x| -> only the top-2 experts are run on
    device, with the gate weight folded into the 1x1 conv weights.

Device (per core, pure data parallel, 1 image per core):
  conv1 3x3 (128->256ch) + per-image top-2 expert subnets (two 3x3 convs,
  elementwise product, 1x1 conv), all as bf16 matmuls with f32 PSUM
  accumulation, channels on partitions, 3x3 conv = 9 shifted-AP matmuls.
"""
import numpy as np
import ml_dtypes

bf16 = ml_dtypes.bfloat16

B, C, H, W = 8, 128, 128, 128
HW = H * W
E, TOPK = 3, 2
N_CORES = 8
Hp, Wp = H + 2, W + 2          # padded image in SBUF
PADN = Hp * Wp                 # 16900
NT = 512                       # pixels per psum tile (4 rows)
RB = HW // NT                  # 32 row-blocks (4 rows each)
XCH = 2048                     # x dma/abs chunk: 16 rows
NXC = HW // XCH                # 8 chunks


# ---------------------------------------------------------------- host math
def _softmax64(z):
    z = z - z.max(axis=-1, keepdims=True)
    e = np.exp(z)
    return e / e.sum(axis=-1, keepdims=True)


def host_gate(x, c1w, c1b, gw):
    """Exact FEM gate from region sums (no conv needed).

    mean_{h,w} conv(t)[o] = (sum_{i,kh,kw} w[o,i,kh,kw] * S[i,kh,kw]
                             + HW*b[o]) / HW
    where S is the sum of t over the tap's valid input region.
    """
    t = np.abs(x).astype(np.float64)                       # [B,C,H,W]
    tot = t.sum(axis=(2, 3))                               # [B,C]
    row = t.sum(axis=3)                                    # [B,C,H]
    col = t.sum(axis=2)                                    # [B,C,W]
    S = np.empty((B, C, 3, 3), np.float64)
    for kh in range(3):
        ex_r = 127 if kh == 0 else (0 if kh == 2 else None)
        for kw in range(3):
            ex_c = 127 if kw == 0 else (0 if kw == 2 else None)
            s = tot.copy()
            if ex_r is not None:
                s -= row[:, :, ex_r]
            if ex_c is not None:
                s -= col[:, :, ex_c]
            if ex_r is not None and ex_c is not None:
                s += t[:, :, ex_r, ex_c]
            S[:, :, kh, kw] = s
    w64 = c1w[:C].astype(np.float64)                       # xh half only
    mean_xh = (np.einsum('oikl,bikl->bo', w64, S, optimize=True)
               + HW * c1b[:C].astype(np.float64)) / HW
    logits = mean_xh @ gw.astype(np.float64).T             # [B,E]
    wts = _softmax64(logits)
    idx = np.argsort(-wts, axis=1, kind='stable')[:, :TOPK]  # top-2, jax order
    g = np.take_along_axis(wts, idx, axis=1)               # [B,2]
    return idx, g.astype(np.float64)


def host_out2(c1b, gw, ew1, eb1, ew2, eb2, ew3, eb3):
    """FEM-b output for zero input: 9-border-class closed form. [C,H,W] f32."""
    xh_c = c1b[:C].astype(np.float64)
    k_c = c1b[C:].astype(np.float64)
    logits = gw.astype(np.float64) @ xh_c
    wts = _softmax64(logits[None])[0]
    idx = np.argsort(-wts, kind='stable')[:TOPK]

    # class index: 0=edge-low, 1=interior, 2=edge-high per axis
    # valid taps for class: low edge (h=0): kh in {1,2}; interior: all;
    # high edge (h=H-1): kh in {0,1}
    def cls_taps(c):
        return {0: (1, 2), 1: (0, 1, 2), 2: (0, 1)}[c]

    o_cls = np.zeros((C, 3, 3), np.float64)
    for e in idx:
        w1s = np.einsum('oikl,i->okl', ew1[e].astype(np.float64), xh_c)
        w2s = np.einsum('oikl,i->okl', ew2[e].astype(np.float64), k_c)
        A = np.zeros((C, 3, 3), np.float64)
        Bv = np.zeros((C, 3, 3), np.float64)
        for ch in range(3):
            for cw_ in range(3):
                A[:, ch, cw_] = (w1s[:, cls_taps(ch), :][:, :, cls_taps(cw_)]
                                 .sum(axis=(1, 2)) + eb1[e].astype(np.float64))
                Bv[:, ch, cw_] = (w2s[:, cls_taps(ch), :][:, :, cls_taps(cw_)]
                                  .sum(axis=(1, 2)) + eb2[e].astype(np.float64))
        M = A * Bv
        w3 = ew3[e][:, :, 0, 0].astype(np.float64)
        o_e = np.einsum('oc,cij->oij', w3, M) + eb3[e].astype(np.float64)[:, None, None]
        o_cls += wts[e] * o_e

    out2_cls = xh_c[:, None, None] + o_cls                 # [C,3,3]
    hcls = np.ones(H, np.intp); hcls[0] = 0; hcls[-1] = 2
    wcls = np.ones(W, np.intp); wcls[0] = 0; wcls[-1] = 2
    out2 = out2_cls[:, hcls][:, :, wcls]                   # [C,H,W]
    return out2.astype(np.float32)


# ---------------------------------------------------------------- bass build
_CACHE = {}


def _build_nc():
    import concourse.bass as bass
    import concourse.mybir as mybir

    f32 = mybir.dt.float32
    bf = mybir.dt.bfloat16
    AF = mybir.ActivationFunctionType
    ALU = mybir.AluOpType

    nc = bass.Bass()
    x_d = nc.declare_dram_parameter("x", [C, HW], f32, isOutput=False)
    cw_d = nc.declare_dram_parameter("cw", [C, 9, 256], bf, isOutput=False)
    ew_d = nc.declare_dram_parameter("ew", [C, 36, C], bf, isOutput=False)
    w3_d = nc.declare_dram_parameter("w3", [C, 2, C], bf, isOutput=False)
    bias_d = nc.declare_dram_parameter("bias", [C, 8], f32, isOutput=False)
    base_d = nc.declare_dram_parameter("base", [C, HW], f32, isOutput=False)
    out_d = nc.declare_dram_parameter("out", [C, HW], f32, isOutput=True)

    from contextlib import ExitStack
    with ExitStack() as _es:
        ec = _es.enter_context
        x_sb = ec(nc.sbuf_tensor([C, HW], f32))
        t_pad = ec(nc.sbuf_tensor([C, PADN], bf))
        xh_pad = ec(nc.sbuf_tensor([C, PADN], bf))
        k_pad = ec(nc.sbuf_tensor([C, PADN], bf))
        cw_sb = ec(nc.sbuf_tensor([C, 9, 256], bf))
        ew_sb = ec(nc.sbuf_tensor([C, 36, C], bf))
        w3_sb = ec(nc.sbuf_tensor([C, 2, C], bf))
        bias_sb = ec(nc.sbuf_tensor([C, 8], f32))
        tmpb_sb = ec(nc.sbuf_tensor([C, NT], bf))
        m_sb = ec(nc.sbuf_tensor([C, 2, 2, NT], bf))     # [expert][depth]
        c_sb = ec(nc.sbuf_tensor([C, 2, NT], f32))       # xh+b3sum staging
        out_sb = ec(nc.sbuf_tensor([C, 2, NT], f32))
        base_sb = ec(nc.sbuf_tensor([C, 2, NT], f32))
        p0 = ec(nc.psum_tensor([C, NT], f32))
        p1 = ec(nc.psum_tensor([C, NT], f32))
        p2 = ec(nc.psum_tensor([C, NT], f32))
        p3 = ec(nc.psum_tensor([C, NT], f32))
        pa0 = ec(nc.psum_tensor([C, NT], f32))
        pb0 = ec(nc.psum_tensor([C, NT], f32))
        pa1 = ec(nc.psum_tensor([C, NT], f32))
        pb1 = ec(nc.psum_tensor([C, NT], f32))
        s_dma_in = ec(nc.semaphore())
        s_abs = ec(nc.semaphore())
        s_border = ec(nc.semaphore())
        s_c1 = ec(nc.semaphore())
        s_c1e = ec(nc.semaphore())
        s_ab = ec(nc.semaphore())
        s_m = ec(nc.semaphore())
        s_o = ec(nc.semaphore())
        s_x = ec(nc.semaphore())
        s_oe = ec(nc.semaphore())
        s_out = ec(nc.semaphore())
        s_dma_out = ec(nc.semaphore())
        block = ec(nc.Block())
        pc1 = [p0, p1, p2, p3]
        pab = [[pa0, pb0], [pa1, pb1]]   # [expert][a/b]
        po = [p0, p1]                    # reused after conv1 fully evicted

        # -------- sync engine: all DMA
        @block.sync
        def _(sync):
            n_in = 0
            # weights
            sync.dma_start(out=cw_sb[:], in_=cw_d[:, :, :]).then_inc(s_dma_in, 16)
            sync.dma_start(out=ew_sb[:], in_=ew_d[:, :, :]).then_inc(s_dma_in, 16)
            sync.dma_start(out=w3_sb[:], in_=w3_d[:, :, :]).then_inc(s_dma_in, 16)
            sync.dma_start(out=bias_sb[:], in_=bias_d[:, :]).then_inc(s_dma_in, 16)
            n_in += 4
            # x chunks
            for c in range(NXC):
                sync.dma_start(
                    out=x_sb[:, c * XCH:(c + 1) * XCH],
                    in_=x_d[:, c * XCH:(c + 1) * XCH],
                ).then_inc(s_dma_in, 16)
                n_in += 1
            # phase B: base blocks + output blocks
            for rb in range(RB):
                if rb >= 2:
                    sync.wait_ge(s_out, rb - 1)      # base_sb slot consumed
                sync.dma_start(
                    out=base_sb[:, rb % 2, :],
                    in_=base_d[:, rb * NT:(rb + 1) * NT],
                ).then_inc(s_dma_in, 16)
                n_in += 1
                sync.wait_ge(s_out, rb + 1)
                sync.dma_start(
                    out=out_d[:, rb * NT:(rb + 1) * NT],
                    in_=out_sb[:, rb % 2, :],
                ).then_inc(s_dma_out, 16)

        # -------- vector engine: border memsets, m products, output combine
        @block.vector
        def _(vector):
            for buf in (t_pad, xh_pad, k_pad):
                a2 = buf[:, :].rearrange("c (h w) -> c h w", h=Hp)
                vector.memset(a2[:, 0, :], 0.0)                 # top row
                vector.memset(a2[:, Hp - 1, :], 0.0)            # bottom row
                vector.memset(a2[:, 1:Hp - 1, 0], 0.0)          # left col
                vector.memset(a2[:, 1:Hp - 1, Wp - 1], 0.0).then_inc(s_border, 1)
            for rb in range(RB):
                for e in range(2):
                    # wait a & b conv groups done (4 groups per rb)
                    vector.wait_ge(s_ab, 4 * rb + 2 * e + 2)
                    # tmpb = b_ps + eb2[e]
                    vector.tensor_scalar(
                        out=tmpb_sb[:, :], in0=pab[e][1][:, :],
                        scalar1=bias_sb[:, 3 + 2 * e:4 + 2 * e], scalar2=None,
                        op0=ALU.add,
                    )
                    # m = (a_ps + eb1[e]) * tmpb
                    vector.scalar_tensor_tensor(
                        out=m_sb[:, e, rb % 2, :], in0=pab[e][0][:, :],
                        scalar=bias_sb[:, 2 + 2 * e:3 + 2 * e],
                        in1=tmpb_sb[:, :],
                        op0=ALU.add, op1=ALU.mult,
                    ).then_inc(s_m, 1)
                if rb >= 1:
                    rp = rb - 1
                    # out = o_ps + c_sb  (o_ps consumed -> s_oe)
                    vector.wait_ge(s_o, rp + 1)
                    vector.wait_ge(s_x, rp + 1)
                    if rp >= 2:
                        vector.wait_ge(s_dma_out, 16 * (rp - 1))
                    vector.scalar_tensor_tensor(
                        out=out_sb[:, rp % 2, :], in0=po[rp % 2][:, :],
                        scalar=0.0, in1=c_sb[:, rp % 2, :],
                        op0=ALU.add, op1=ALU.add,
                    ).then_inc(s_oe, 1)
                    # out += base
                    vector.wait_ge(s_dma_in, 16 * (12 + rp + 1))
                    vector.scalar_tensor_tensor(
                        out=out_sb[:, rp % 2, :], in0=out_sb[:, rp % 2, :],
                        scalar=0.0, in1=base_sb[:, rp % 2, :],
                        op0=ALU.add, op1=ALU.add,
                    ).then_inc(s_out, 1)
            # final block
            rp = RB - 1
            vector.wait_ge(s_o, rp + 1)
            vector.wait_ge(s_x, rp + 1)
            vector.wait_ge(s_dma_out, 16 * (rp - 1))
            vector.scalar_tensor_tensor(
                out=out_sb[:, rp % 2, :], in0=po[rp % 2][:, :],
                scalar=0.0, in1=c_sb[:, rp % 2, :],
                op0=ALU.add, op1=ALU.add,
            ).then_inc(s_oe, 1)
            vector.wait_ge(s_dma_in, 16 * (12 + rp + 1))
            vector.scalar_tensor_tensor(
                out=out_sb[:, rp % 2, :], in0=out_sb[:, rp % 2, :],
                scalar=0.0, in1=base_sb[:, rp % 2, :],
                op0=ALU.add, op1=ALU.add,
            ).then_inc(s_out, 1)

        # -------- scalar engine: abs casts, conv1 evictions, xh staging
        @block.scalar
        def _(scalar):
            def abs_chunk(c):
                scalar.wait_ge(s_dma_in, 16 * (4 + c + 1))
                t2 = t_pad[:, :].rearrange("c (h w) -> c h w", h=Hp)
                scalar.activation(
                    out=t2[:, 16 * c + 1:16 * c + 17, 1:1 + W],
                    in_=x_sb[:, c * XCH:(c + 1) * XCH].rearrange(
                        "c (h w) -> c h w", h=16),
                    func=AF.Abs,
                ).then_inc(s_abs, 1)

            abs_chunk(0)
            abs_chunk(1)
            nxt = 2
            for rb in range(RB):
                if rb % 2 == 0 and nxt < NXC:
                    abs_chunk(nxt)
                    nxt += 1
                for h in range(2):
                    i = 2 * rb + h
                    scalar.wait_ge(s_c1, i + 1)
                    dst = (xh_pad if h == 0 else k_pad)[:, :].rearrange(
                        "c (h w) -> c h w", h=Hp)
                    scalar.activation(
                        out=dst[:, 4 * rb + 1:4 * rb + 5, 1:1 + W],
                        in_=pc1[i % 4][:, :],
                        func=AF.Identity,
                        bias=bias_sb[:, h:h + 1], scale=1.0,
                    ).then_inc(s_c1e, 1)
            # phase B: c_sb = xh_interior + b3sum
            for rb in range(RB):
                if rb >= 2:
                    scalar.wait_ge(s_oe, rb - 1)     # c_sb slot consumed
                xh2 = xh_pad[:, :].rearrange("c (h w) -> c h w", h=Hp)
                scalar.activation(
                    out=c_sb[:, rb % 2, :],
                    in_=xh2[:, 4 * rb + 1:4 * rb + 5, 1:1 + W],
                    func=AF.Identity,
                    bias=bias_sb[:, 6:7], scale=1.0,
                ).then_inc(s_x, 1)

        # -------- tensor engine: all matmuls
        @block.tensor
        def _(tensor):
            t2 = t_pad[:, :].rearrange("c (h w) -> c h w", h=Hp)
            xh2 = xh_pad[:, :].rearrange("c (h w) -> c h w", h=Hp)
            k2 = k_pad[:, :].rearrange("c (h w) -> c h w", h=Hp)
            tensor.wait_ge(s_border, 1)        # t_pad borders zeroed
            tensor.wait_ge(s_dma_in, 16)       # cw loaded
            # conv1
            for rb in range(RB):
                c_need = min(NXC - 1, (4 * rb + 4) // 16)
                tensor.wait_ge(s_abs, c_need + 1)
                for h in range(2):
                    i = 2 * rb + h
                    if i >= 4:
                        tensor.wait_ge(s_c1e, i - 3)
                    for tap in range(9):
                        kh, kw = divmod(tap, 3)
                        mm = tensor.matmul(
                            pc1[i % 4][:, :],
                            cw_sb[:, tap, 128 * h:128 * h + 128],
                            t2[:, 4 * rb + kh:4 * rb + kh + 4, kw:kw + W],
                            start=(tap == 0), stop=(tap == 8),
                        )
                    mm.then_inc(s_c1, 1)
            # experts (ew_sb groups: (e, j, tap) -> 18*e + 9*j + tap)
            tensor.wait_ge(s_border, 3)
            tensor.wait_ge(s_dma_in, 16 * 4)
            for rb in range(RB):
                for e in range(2):
                    for j in range(2):         # j=0: a from xh, j=1: b from k
                        if rb >= 1:
                            tensor.wait_ge(s_m, 2 * (rb - 1) + e + 1)
                        tensor.wait_ge(s_c1e, min(2 * RB, 2 * rb + 4))
                        src = xh2 if j == 0 else k2
                        for tap in range(9):
                            kh, kw = divmod(tap, 3)
                            mm = tensor.matmul(
                                pab[e][j][:, :],
                                ew_sb[:, 18 * e + 9 * j + tap, :],
                                src[:, 4 * rb + kh:4 * rb + kh + 4, kw:kw + W],
                                start=(tap == 0), stop=(tap == 8),
                            )
                        mm.then_inc(s_ab, 1)
                if rb >= 1:
                    rp = rb - 1
                    tensor.wait_ge(s_m, 2 * rp + 2)
                    tensor.wait_ge(s_c1e, 2 * RB)
                    if rp >= 2:
                        tensor.wait_ge(s_oe, rp - 1)
                    for e in range(2):
                        mm = tensor.matmul(
                            po[rp % 2][:, :],
                            w3_sb[:, e, :],
                            m_sb[:, e, rp % 2, :],
                            start=(e == 0), stop=(e == 1),
                        )
                    mm.then_inc(s_o, 1)
            rp = RB - 1
            tensor.wait_ge(s_m, 2 * rp + 2)
            tensor.wait_ge(s_oe, rp - 1)
            for e in range(2):
                mm = tensor.matmul(
                    po[rp % 2][:, :], w3_sb[:, e, :], m_sb[:, e, rp % 2, :],
                    start=(e == 0), stop=(e == 1),
                )
            mm.then_inc(s_o, 1)

    return nc


def _get_nc():
    if "nc" not in _CACHE:
        _CACHE["nc"] = _build_nc()
    return _CACHE["nc"]


# ---------------------------------------------------------------- packing
def _pack_inputs(inputs):
    x = inputs["x"].astype(np.float32)
    idx, g = host_gate(x, inputs["a_c1w"], inputs["a_c1b"], inputs["a_gw"])
    out2 = host_out2(inputs["b_c1b"], inputs["b_gw"],
                     inputs["b_ew1"], inputs["b_eb1"],
                     inputs["b_ew2"], inputs["b_eb2"],
                     inputs["b_ew3"], inputs["b_eb3"])     # [C,H,W] f32

    # conv1 weights -> [C(ci), 9(tap), 256(co)] bf16
    cw = np.ascontiguousarray(
        inputs["a_c1w"].reshape(256, C, 9).transpose(1, 2, 0)).astype(bf16)

    in_maps = []
    for core in range(N_CORES):
        e0, e1 = int(idx[core, 0]), int(idx[core, 1])
        g0, g1 = float(g[core, 0]), float(g[core, 1])
        # expert taps -> [C(ci), 36(e,j,tap), C(co)]
        ew = np.empty((C, 2 * 2 * 9, C), np.float32)
        for s, e in enumerate((e0, e1)):
            w1 = inputs["a_ew1"][e].reshape(C, C, 9).transpose(1, 2, 0)
            w2 = inputs["a_ew2"][e].reshape(C, C, 9).transpose(1, 2, 0)
            ew[:, 18 * s:18 * s + 9, :] = w1
            ew[:, 18 * s + 9:18 * s + 18, :] = w2
        w3 = np.empty((C, 2, C), np.float32)
        w3[:, 0, :] = inputs["a_ew3"][e0][:, :, 0, 0].T * g0
        w3[:, 1, :] = inputs["a_ew3"][e1][:, :, 0, 0].T * g1
        bias = np.zeros((C, 8), np.float32)
        bias[:, 0] = inputs["a_c1b"][:C]
        bias[:, 1] = inputs["a_c1b"][C:]
        bias[:, 2] = inputs["a_eb1"][e0]
        bias[:, 3] = inputs["a_eb2"][e0]
        bias[:, 4] = inputs["a_eb1"][e1]
        bias[:, 5] = inputs["a_eb2"][e1]
        bias[:, 6] = g0 * inputs["a_eb3"][e0] + g1 * inputs["a_eb3"][e1]
        base = (out2 + x[core]).reshape(C, HW)
        in_maps.append({
            "x": np.ascontiguousarray(x[core].reshape(C, HW)),
            "cw": cw,
            "ew": ew.astype(bf16),
            "w3": w3.astype(bf16),
            "bias": bias,
            "base": np.ascontiguousarray(base.astype(np.float32)),
        })
    return in_maps


# ---------------------------------------------------------------- entry
def _enable_jax_cache():
    # persistent jax compile cache hung under the axon backend; disabled.
    _CACHE["jax_cache_set"] = True


def kernel(**inputs) -> np.ndarray:
    from concourse.bass_utils import run_bass_kernel_spmd

    _enable_jax_cache()
    nc = _get_nc()
    in_maps = _pack_inputs(inputs)
    res = run_bass_kernel_spmd(nc, in_maps, list(range(N_CORES)))
    out = np.stack([res.results[c]["out"].reshape(C, H, W)
                    for c in range(N_CORES)], axis=0)
    return out.astype(np.float32)


# revision 6
# speedup vs baseline: 36.0820x; 36.0820x over previous
"""Trainium2 kernel for nn_Dec_module_74285754351968 (FD_FEM: fft-split + MoE convs).

Mathematical structure exploited:
  - fd_split's fft/ifft roundtrip cancels exactly: high = |x|, low = 0.
  - FEM-b therefore sees an all-zero input -> its output is batch-independent
    and piecewise-constant over 9 border classes; computed on host in closed
    form (no convs needed).
  - FEM-a's gate (softmax over spatial means of conv1 output) is computed on
    host exactly from region sums of |x| -> only the top-2 experts are run on
    device, with the gate weight folded into the 1x1 conv weights.

Device (per core, pure data parallel, 1 image per core):
  conv1 3x3 (128->256ch) + per-image top-2 expert subnets (two 3x3 convs,
  elementwise product, 1x1 conv), all as bf16 matmuls with f32 PSUM
  accumulation, channels on partitions, 3x3 conv = 9 shifted-AP matmuls.
"""
import numpy as np
import ml_dtypes

bf16 = ml_dtypes.bfloat16

B, C, H, W = 8, 128, 128, 128
HW = H * W
E, TOPK = 3, 2
N_CORES = 8
Hp, Wp = H + 2, W + 2          # padded image in SBUF
PADN = Hp * Wp                 # 16900
NT = 512                       # pixels per psum tile (4 rows)
RB = HW // NT                  # 32 row-blocks (4 rows each)
XCH = 2048                     # x dma/abs chunk: 16 rows
NXC = HW // XCH                # 8 chunks


# ---------------------------------------------------------------- host math
def _softmax64(z):
    z = z - z.max(axis=-1, keepdims=True)
    e = np.exp(z)
    return e / e.sum(axis=-1, keepdims=True)


def host_gate(x, c1w, c1b, gw):
    """Exact FEM gate from region sums (no conv needed).

    mean_{h,w} conv(t)[o] = (sum_{i,kh,kw} w[o,i,kh,kw] * S[i,kh,kw]
                             + HW*b[o]) / HW
    where S is the sum of t over the tap's valid input region.
    """
    t = np.abs(x).astype(np.float64)                       # [B,C,H,W]
    tot = t.sum(axis=(2, 3))                               # [B,C]
    row = t.sum(axis=3)                                    # [B,C,H]
    col = t.sum(axis=2)                                    # [B,C,W]
    S = np.empty((B, C, 3, 3), np.float64)
    for kh in range(3):
        ex_r = 127 if kh == 0 else (0 if kh == 2 else None)
        for kw in range(3):
            ex_c = 127 if kw == 0 else (0 if kw == 2 else None)
            s = tot.copy()
            if ex_r is not None:
                s -= row[:, :, ex_r]
            if ex_c is not None:
                s -= col[:, :, ex_c]
            if ex_r is not None and ex_c is not None:
                s += t[:, :, ex_r, ex_c]
            S[:, :, kh, kw] = s
    w64 = c1w[:C].astype(np.float64)                       # xh half only
    mean_xh = (np.einsum('oikl,bikl->bo', w64, S, optimize=True)
               + HW * c1b[:C].astype(np.float64)) / HW
    logits = mean_xh @ gw.astype(np.float64).T             # [B,E]
    wts = _softmax64(logits)
    idx = np.argsort(-wts, axis=1, kind='stable')[:, :TOPK]  # top-2, jax order
    g = np.take_along_axis(wts, idx, axis=1)               # [B,2]
    return idx, g.astype(np.float64)


def host_out2(c1b, gw, ew1, eb1, ew2, eb2, ew3, eb3):
    """FEM-b output for zero input: 9-border-class closed form. [C,H,W] f32."""
    xh_c = c1b[:C].astype(np.float64)
    k_c = c1b[C:].astype(np.float64)
    logits = gw.astype(np.float64) @ xh_c
    wts = _softmax64(logits[None])[0]
    idx = np.argsort(-wts, kind='stable')[:TOPK]

    # class index: 0=edge-low, 1=interior, 2=edge-high per axis
    # valid taps for class: low edge (h=0): kh in {1,2}; interior: all;
    # high edge (h=H-1): kh in {0,1}
    def cls_taps(c):
        return {0: (1, 2), 1: (0, 1, 2), 2: (0, 1)}[c]

    o_cls = np.zeros((C, 3, 3), np.float64)
    for e in idx:
        w1s = np.einsum('oikl,i->okl', ew1[e].astype(np.float64), xh_c)
        w2s = np.einsum('oikl,i->okl', ew2[e].astype(np.float64), k_c)
        A = np.zeros((C, 3, 3), np.float64)
        Bv = np.zeros((C, 3, 3), np.float64)
        for ch in range(3):
            for cw_ in range(3):
                A[:, ch, cw_] = (w1s[:, cls_taps(ch), :][:, :, cls_taps(cw_)]
                                 .sum(axis=(1, 2)) + eb1[e].astype(np.float64))
                Bv[:, ch, cw_] = (w2s[:, cls_taps(ch), :][:, :, cls_taps(cw_)]
                                  .sum(axis=(1, 2)) + eb2[e].astype(np.float64))
        M = A * Bv
        w3 = ew3[e][:, :, 0, 0].astype(np.float64)
        o_e = np.einsum('oc,cij->oij', w3, M) + eb3[e].astype(np.float64)[:, None, None]
        o_cls += wts[e] * o_e

    out2_cls = xh_c[:, None, None] + o_cls                 # [C,3,3]
    hcls = np.ones(H, np.intp); hcls[0] = 0; hcls[-1] = 2
    wcls = np.ones(W, np.intp); wcls[0] = 0; wcls[-1] = 2
    out2 = out2_cls[:, hcls][:, :, wcls]                   # [C,H,W]
    return out2.astype(np.float32)


# ---------------------------------------------------------------- bass build
_CACHE = {}


def _build_nc(reps=1):
    import concourse.bass as bass
    import concourse.mybir as mybir

    f32 = mybir.dt.float32
    bf = mybir.dt.bfloat16
    AF = mybir.ActivationFunctionType
    ALU = mybir.AluOpType

    nc = bass.Bass()
    x_d = nc.declare_dram_parameter("x", [C, HW], f32, isOutput=False)
    cw_d = nc.declare_dram_parameter("cw", [C, 9, 256], bf, isOutput=False)
    ew_d = nc.declare_dram_parameter("ew", [C, 36, C], bf, isOutput=False)
    w3_d = nc.declare_dram_parameter("w3", [C, 2, C], bf, isOutput=False)
    bias_d = nc.declare_dram_parameter("bias", [C, 8], f32, isOutput=False)
    base_d = nc.declare_dram_parameter("base", [C, HW], f32, isOutput=False)
    out_d = nc.declare_dram_parameter("out", [C, HW], f32, isOutput=True)

    from contextlib import ExitStack
    with ExitStack() as _es:
        ec = _es.enter_context
        x_sb = ec(nc.sbuf_tensor([C, HW], f32))
        t_pad = ec(nc.sbuf_tensor([C, PADN], bf))
        xh_pad = ec(nc.sbuf_tensor([C, PADN], bf))
        k_pad = ec(nc.sbuf_tensor([C, PADN], bf))
        cw_sb = ec(nc.sbuf_tensor([C, 9, 256], bf))
        ew_sb = ec(nc.sbuf_tensor([C, 36, C], bf))
        w3_sb = ec(nc.sbuf_tensor([C, 2, C], bf))
        bias_sb = ec(nc.sbuf_tensor([C, 8], f32))
        tmpb_sb = ec(nc.sbuf_tensor([C, NT], bf))
        m_sb = ec(nc.sbuf_tensor([C, 2, 2, NT], bf))     # [expert][depth]
        c_sb = ec(nc.sbuf_tensor([C, 2, NT], f32))       # xh+b3sum staging
        out_sb = ec(nc.sbuf_tensor([C, 2, NT], f32))
        base_sb = ec(nc.sbuf_tensor([C, 2, NT], f32))
        p0 = ec(nc.psum_tensor([C, NT], f32))
        p1 = ec(nc.psum_tensor([C, NT], f32))
        p2 = ec(nc.psum_tensor([C, NT], f32))
        p3 = ec(nc.psum_tensor([C, NT], f32))
        pa0 = ec(nc.psum_tensor([C, NT], f32))
        pb0 = ec(nc.psum_tensor([C, NT], f32))
        pa1 = ec(nc.psum_tensor([C, NT], f32))
        pb1 = ec(nc.psum_tensor([C, NT], f32))
        s_dma_x = ec(nc.semaphore())
        s_dma_w = ec(nc.semaphore())
        s_dma_b = ec(nc.semaphore())
        s_abs = ec(nc.semaphore())
        s_border = ec(nc.semaphore())
        s_c1 = ec(nc.semaphore())
        s_c1e = ec(nc.semaphore())
        s_ab = ec(nc.semaphore())
        s_m = ec(nc.semaphore())
        s_o = ec(nc.semaphore())
        s_x = ec(nc.semaphore())
        s_oe = ec(nc.semaphore())
        s_out = ec(nc.semaphore())
        s_dma_out = ec(nc.semaphore())
        block = ec(nc.Block())
        pc1 = [p0, p1, p2, p3]
        pab = [[pa0, pb0], [pa1, pb1]]   # [expert][a/b]
        po = [p0, p1]                    # reused after conv1 fully evicted

        def r3(buf):
            return buf[:, :].rearrange("c (h w) -> c h w", h=Hp)

        # -------- sync engine: all DMA
        @block.sync
        def _(sync):
            for r in range(reps):
                # x chunk 0 first (gates the first abs), then conv1 weights
                for c in range(NXC):
                    if r >= 1:
                        sync.wait_ge(s_abs, (r - 1) * 8 + c + 1)
                    sync.dma_start(
                        out=x_sb[:, c * XCH:(c + 1) * XCH],
                        in_=x_d[:, c * XCH:(c + 1) * XCH],
                    ).then_inc(s_dma_x, 16)
                    if r == 0 and c == 0:
                        sync.dma_start(out=cw_sb[:], in_=cw_d[:, :, :]
                                       ).then_inc(s_dma_w, 16)
                        sync.dma_start(out=bias_sb[:], in_=bias_d[:, :]
                                       ).then_inc(s_dma_w, 16)
                        sync.dma_start(out=ew_sb[:], in_=ew_d[:, :, :]
                                       ).then_inc(s_dma_w, 16)
                        sync.dma_start(out=w3_sb[:], in_=w3_d[:, :, :]
                                       ).then_inc(s_dma_w, 16)
                # phase B: base blocks + output blocks
                for rb in range(RB):
                    G = r * RB + rb
                    if G >= 2:
                        sync.wait_ge(s_out, G - 1)     # base_sb slot consumed
                    sync.dma_start(
                        out=base_sb[:, rb % 2, :],
                        in_=base_d[:, rb * NT:(rb + 1) * NT],
                    ).then_inc(s_dma_b, 16)
                    sync.wait_ge(s_out, G + 1)
                    sync.dma_start(
                        out=out_d[:, rb * NT:(rb + 1) * NT],
                        in_=out_sb[:, rb % 2, :],
                    ).then_inc(s_dma_out, 16)

        # -------- vector engine: border memsets, m products, output combine
        @block.vector
        def _(vector):
            for buf in (t_pad, xh_pad, k_pad):
                a2 = r3(buf)
                vector.memset(a2[:, 0, :], 0.0)                 # top row
                vector.memset(a2[:, Hp - 1, :], 0.0)            # bottom row
                vector.memset(a2[:, 1:Hp - 1, 0], 0.0)          # left col
                vector.memset(a2[:, 1:Hp - 1, Wp - 1], 0.0).then_inc(s_border, 1)

            def combine(r, rp):
                G = r * RB + rp
                # out = o_ps + c_sb  (o_ps consumed -> s_oe)
                vector.wait_ge(s_o, G + 1)
                vector.wait_ge(s_x, G + 1)
                if G >= 2:
                    vector.wait_ge(s_dma_out, 16 * (G - 1))
                vector.scalar_tensor_tensor(
                    out=out_sb[:, rp % 2, :], in0=po[rp % 2][:, :],
                    scalar=0.0, in1=c_sb[:, rp % 2, :],
                    op0=ALU.add, op1=ALU.add,
                ).then_inc(s_oe, 1)
                # out += base
                vector.wait_ge(s_dma_b, 16 * (G + 1))
                vector.scalar_tensor_tensor(
                    out=out_sb[:, rp % 2, :], in0=out_sb[:, rp % 2, :],
                    scalar=0.0, in1=base_sb[:, rp % 2, :],
                    op0=ALU.add, op1=ALU.add,
                ).then_inc(s_out, 1)

            for r in range(reps):
                for rb in range(RB):
                    G = r * RB + rb
                    for e in range(2):
                        # wait a & b conv groups done (4 groups per rb)
                        vector.wait_ge(s_ab, 4 * G + 2 * e + 2)
                        # tmpb = b_ps + eb2[e]
                        vector.tensor_scalar(
                            out=tmpb_sb[:, :], in0=pab[e][1][:, :],
                            scalar1=bias_sb[:, 3 + 2 * e:4 + 2 * e],
                            scalar2=None, op0=ALU.add,
                        )
                        # m = (a_ps + eb1[e]) * tmpb
                        vector.scalar_tensor_tensor(
                            out=m_sb[:, e, rb % 2, :], in0=pab[e][0][:, :],
                            scalar=bias_sb[:, 2 + 2 * e:3 + 2 * e],
                            in1=tmpb_sb[:, :],
                            op0=ALU.add, op1=ALU.mult,
                        ).then_inc(s_m, 1)
                    if rb >= 1:
                        combine(r, rb - 1)
                combine(r, RB - 1)

        # -------- scalar engine: abs casts, conv1 evictions, xh staging
        @block.scalar
        def _(scalar):
            t2s = r3(t_pad)

            def abs_chunk(r, c):
                scalar.wait_ge(s_dma_x, r * 128 + 16 * (c + 1))
                scalar.activation(
                    out=t2s[:, 16 * c + 1:16 * c + 17, 1:1 + W],
                    in_=x_sb[:, c * XCH:(c + 1) * XCH].rearrange(
                        "c (h w) -> c h w", h=16),
                    func=AF.Abs,
                ).then_inc(s_abs, 1)

            for r in range(reps):
                abs_chunk(r, 0)
                abs_chunk(r, 1)
                nxt = 2
                for rb in range(RB):
                    if rb % 2 == 0 and nxt < NXC:
                        abs_chunk(r, nxt)
                        nxt += 1
                    for h in range(2):
                        i = 2 * rb + h
                        scalar.wait_ge(s_c1, r * 64 + i + 1)
                        dst = r3(xh_pad if h == 0 else k_pad)
                        scalar.activation(
                            out=dst[:, 4 * rb + 1:4 * rb + 5, 1:1 + W],
                            in_=pc1[i % 4][:, :],
                            func=AF.Identity,
                            bias=bias_sb[:, h:h + 1], scale=1.0,
                        ).then_inc(s_c1e, 1)
                # phase B: c_sb = xh_interior + b3sum
                xh2 = r3(xh_pad)
                for rb in range(RB):
                    G = r * RB + rb
                    if G >= 2:
                        scalar.wait_ge(s_oe, G - 1)    # c_sb slot consumed
                    scalar.activation(
                        out=c_sb[:, rb % 2, :],
                        in_=xh2[:, 4 * rb + 1:4 * rb + 5, 1:1 + W],
                        func=AF.Identity,
                        bias=bias_sb[:, 6:7], scale=1.0,
                    ).then_inc(s_x, 1)

        # -------- tensor engine: all matmuls
        @block.tensor
        def _(tensor):
            t2 = r3(t_pad)
            xh2 = r3(xh_pad)
            k2 = r3(k_pad)
            tensor.wait_ge(s_border, 3)
            tensor.wait_ge(s_dma_w, 64)        # all weights loaded
            for r in range(reps):
                # conv1
                for rb in range(RB):
                    c_need = min(NXC - 1, (4 * rb + 4) // 16)
                    tensor.wait_ge(s_abs, r * 8 + c_need + 1)
                    for h in range(2):
                        i = 2 * rb + h
                        I = r * 64 + i
                        if I >= 4:
                            tensor.wait_ge(s_c1e, I - 3)
                        if r >= 1 and i == 0:
                            tensor.wait_ge(s_oe, r * RB)   # P0/P1 freed
                        for tap in range(9):
                            kh, kw = divmod(tap, 3)
                            mm = tensor.matmul(
                                pc1[i % 4][:, :],
                                cw_sb[:, tap, 128 * h:128 * h + 128],
                                t2[:, 4 * rb + kh:4 * rb + kh + 4, kw:kw + W],
                                start=(tap == 0), stop=(tap == 8),
                            )
                        mm.then_inc(s_c1, 1)

                def o_mm(rp):
                    G = r * RB + rp
                    tensor.wait_ge(s_m, 2 * G + 2)
                    tensor.wait_ge(s_c1e, r * 64 + 64)
                    if G >= 2:
                        tensor.wait_ge(s_oe, G - 1)
                    for e in range(2):
                        mm = tensor.matmul(
                            po[rp % 2][:, :], w3_sb[:, e, :],
                            m_sb[:, e, rp % 2, :],
                            start=(e == 0), stop=(e == 1),
                        )
                    mm.then_inc(s_o, 1)

                # experts (ew_sb groups: (e, j, tap) -> 18*e + 9*j + tap)
                for rb in range(RB):
                    G = r * RB + rb
                    for e in range(2):
                        for j in range(2):     # j=0: a from xh, j=1: b from k
                            if G >= 1:
                                tensor.wait_ge(s_m, 2 * (G - 1) + e + 1)
                            tensor.wait_ge(s_c1e,
                                           r * 64 + min(2 * RB, 2 * rb + 4))
                            src = xh2 if j == 0 else k2
                            for tap in range(9):
                                kh, kw = divmod(tap, 3)
                                mm = tensor.matmul(
                                    pab[e][j][:, :],
                                    ew_sb[:, 18 * e + 9 * j + tap, :],
                                    src[:, 4 * rb + kh:4 * rb + kh + 4,
                                        kw:kw + W],
                                    start=(tap == 0), stop=(tap == 8),
                                )
                            mm.then_inc(s_ab, 1)
                    if rb >= 1:
                        o_mm(rb - 1)
                o_mm(RB - 1)

    return nc


def _get_nc():
    if "nc" not in _CACHE:
        _CACHE["nc"] = _build_nc()
    return _CACHE["nc"]


# ---------------------------------------------------------------- packing
def _pack_inputs(inputs):
    x = inputs["x"].astype(np.float32)
    idx, g = host_gate(x, inputs["a_c1w"], inputs["a_c1b"], inputs["a_gw"])
    out2 = host_out2(inputs["b_c1b"], inputs["b_gw"],
                     inputs["b_ew1"], inputs["b_eb1"],
                     inputs["b_ew2"], inputs["b_eb2"],
                     inputs["b_ew3"], inputs["b_eb3"])     # [C,H,W] f32

    # conv1 weights -> [C(ci), 9(tap), 256(co)] bf16
    cw = np.ascontiguousarray(
        inputs["a_c1w"].reshape(256, C, 9).transpose(1, 2, 0)).astype(bf16)

    in_maps = []
    for core in range(N_CORES):
        e0, e1 = int(idx[core, 0]), int(idx[core, 1])
        g0, g1 = float(g[core, 0]), float(g[core, 1])
        # expert taps -> [C(ci), 36(e,j,tap), C(co)]
        ew = np.empty((C, 2 * 2 * 9, C), np.float32)
        for s, e in enumerate((e0, e1)):
            w1 = inputs["a_ew1"][e].reshape(C, C, 9).transpose(1, 2, 0)
            w2 = inputs["a_ew2"][e].reshape(C, C, 9).transpose(1, 2, 0)
            ew[:, 18 * s:18 * s + 9, :] = w1
            ew[:, 18 * s + 9:18 * s + 18, :] = w2
        w3 = np.empty((C, 2, C), np.float32)
        w3[:, 0, :] = inputs["a_ew3"][e0][:, :, 0, 0].T * g0
        w3[:, 1, :] = inputs["a_ew3"][e1][:, :, 0, 0].T * g1
        bias = np.zeros((C, 8), np.float32)
        bias[:, 0] = inputs["a_c1b"][:C]
        bias[:, 1] = inputs["a_c1b"][C:]
        bias[:, 2] = inputs["a_eb1"][e0]
        bias[:, 3] = inputs["a_eb2"][e0]
        bias[:, 4] = inputs["a_eb1"][e1]
        bias[:, 5] = inputs["a_eb2"][e1]
        bias[:, 6] = g0 * inputs["a_eb3"][e0] + g1 * inputs["a_eb3"][e1]
        base = (out2 + x[core]).reshape(C, HW)
        in_maps.append({
            "x": np.ascontiguousarray(x[core].reshape(C, HW)),
            "cw": cw,
            "ew": ew.astype(bf16),
            "w3": w3.astype(bf16),
            "bias": bias,
            "base": np.ascontiguousarray(base.astype(np.float32)),
        })
    return in_maps


# ---------------------------------------------------------------- entry
def _enable_jax_cache():
    # persistent jax compile cache hung under the axon backend; disabled.
    _CACHE["jax_cache_set"] = True


def kernel(**inputs) -> np.ndarray:
    from concourse.bass_utils import run_bass_kernel_spmd

    _enable_jax_cache()
    nc = _get_nc()
    in_maps = _pack_inputs(inputs)
    res = run_bass_kernel_spmd(nc, in_maps, list(range(N_CORES)))
    out = np.stack([res.results[c]["out"].reshape(C, H, W)
                    for c in range(N_CORES)], axis=0)
    return out.astype(np.float32)
